# revision 5
# baseline (speedup 1.0000x reference)
"""AttentionPool Trainium2 kernel (8 NeuronCores, SPMD, no collectives).

Math (exactly equivalent to the reference up to fp reordering):
    w_i   = silu(h_i @ W1 + b1) @ W2          (b2 cancels)
    num_g = sum_{i in g} h_i * exp(w_i)
    den_g = sum_{i in g} exp(w_i)
    out_g = num_g / (den_g + eps * exp(max_j w_j))

v2 design (per 768-node group, 82 groups/core):
    mm1:  u[256, n] via fp8e4m3 DoubleRow matmuls, K = 2 k-tiles x 65
          partitions (64 features + a ones-row that carries b1 in the
          stationary, two-level-quantized across the two k-tile slots)
    silu: ONE ACT instruction [128, 1536] PSUM->SBUF fp16 (bias folded)
    mm2:  per-tile stationary-s matmuls, fp16 (12 tiny matmuls/group)
    exp:  e = silu(w)/(w - silu(w)) == exp(w), batched over 4 groups,
          sub/mult/copy offloaded to GpSimd, recip on DVE
    S:    S[p, c] = e_p * (batch_rel_p == c)   (DVE tensor_tensor)
    seg:  num[feat, g] += hN_tile^T @ S        (PE, accumulated in PSUM)

DMA: hN fp16 (sync queue) + hT fp8 (gpsimd queue) + mask fp16 -> ~25MB/core
(vs 34MB baseline), split across two hw DMA queues.

Host: shards nodes at graph boundaries (512 graphs/core), computes den/max/
final divide from the returned per-node logits w.
"""

import math

import ml_dtypes
import numpy as np

NCORES = 8
G_TOTAL = 4096
G_PER_CORE = G_TOTAL // NCORES  # 512
IN_DIM = 128
HID = 256
EPS = 1e-6
GROUP_NODES = 768
TILE_NODES = 128
TPG = GROUP_NODES // TILE_NODES  # 6
NUM_BANK_COLS = 512  # one PSUM bank of f32
EXP_BATCH = 4  # groups per exp-trick batch

BF16 = ml_dtypes.bfloat16
FP16 = np.float16
FP8 = ml_dtypes.float8_e4m3


def _tilepart(a, nt):
    return a.reshape(nt, TILE_NODES, IN_DIM).transpose(1, 0, 2).reshape(
        TILE_NODES, nt * IN_DIM
    )


def _build_host_data(h, batch, W1, b1, W2):
    """Shard at graph boundaries; build per-core arrays + global window plan."""
    batch = np.asarray(batch).astype(np.int64)
    cnt = np.bincount(batch, minlength=G_TOTAL)
    cum = np.concatenate([[0], np.cumsum(cnt)])
    bounds = [int(cum[G_PER_CORE * c]) for c in range(NCORES + 1)]
    sizes = np.diff(bounds)
    npad = int(math.ceil(max(sizes) / GROUP_NODES) * GROUP_NODES)
    nt = npad // TILE_NODES

    # Global (core-invariant) window starts: c0[t] = min over cores of the
    # first graph (relative) in tile t; SPAN covers the max extent.
    lo = np.full(nt, 1 << 30, dtype=np.int64)
    hi = np.full(nt, -1, dtype=np.int64)
    grels = []
    for c in range(NCORES):
        n0, n1 = bounds[c], bounds[c + 1]
        grel = batch[n0:n1] - G_PER_CORE * c
        grels.append(grel)
        ntc = (n1 - n0 + TILE_NODES - 1) // TILE_NODES
        for t in range(ntc):
            seg = grel[TILE_NODES * t : TILE_NODES * t + TILE_NODES]
            lo[t] = min(lo[t], int(seg[0]))
            hi[t] = max(hi[t], int(seg[-1]))
    span = 8
    while span < int(max(hi - lo)) + 1:
        span *= 2
    assert span <= 64, f"window span {span} unexpectedly large"
    c0 = np.where(hi >= 0, lo, 0).astype(np.int64)
    c0 = np.minimum(c0, G_PER_CORE - 1)  # clamp (padding tiles)
    # fill padding tiles' c0 with last valid to keep windows sane
    last = 0
    for t in range(nt):
        if hi[t] >= 0:
            last = c0[t]
        else:
            c0[t] = last
    wdt = np.minimum(span, G_PER_CORE - c0).astype(np.int64)  # clip to 512

    per_core = []
    for c in range(NCORES):
        n0, n1 = bounds[c], bounds[c + 1]
        nc_nodes = n1 - n0
        hc = np.empty((npad, IN_DIM), np.float32)
        hc[:nc_nodes] = h[n0:n1]
        hc[nc_nodes:] = h[n0]  # replicate a real node into padding
        # hT fp8 DoubleRow layout: [65, 2*npad], k-tile kt holds feats
        # kt*64 + p on partition p; partition 64 is the ones-row (bias).
        ht = np.ascontiguousarray(hc.T)  # [128, npad] fp32
        hT8 = np.empty((65, 2 * npad), np.float32)
        hT8[0:64, 0:npad] = ht[0:64]
        hT8[0:64, npad:] = ht[64:128]
        hT8[64, :] = 1.0
        brel = np.full(npad, -1000.0, np.float32)
        g = grels[c].astype(np.float32)
        tidx = np.arange(nc_nodes) // TILE_NODES
        brel[:nc_nodes] = g - c0[tidx]
        per_core.append(
            dict(
                hN=_tilepart(hc.astype(FP16), nt),
                hT=hT8.astype(FP8),
                hmask=np.ascontiguousarray(
                    (
                        brel.reshape(nt, TILE_NODES).T[:, :, None]
                        == np.arange(span, dtype=np.float32)[None, None, :]
                    )
                    .astype(FP16)
                    .reshape(TILE_NODES, nt * span)
                ),
                n_nodes=nc_nodes,
                grel=grels[c],
            )
        )

    # W1 fp8 DoubleRow stationary: [65, 512], free = kt*256 + hid.
    # Row 64 carries b1 two-level-quantized: kt0 slot the fp8 rounding of
    # b1, kt1 slot the fp8 rounding of the residual.
    w1f = np.asarray(W1).astype(np.float32)  # [128, 256]
    b1f = np.asarray(b1).reshape(HID).astype(np.float32)
    b1q = b1f.astype(FP8).astype(np.float32)
    b1r = b1f - b1q
    w1dr = np.zeros((65, 2 * HID), np.float32)
    for kt in range(2):
        w1dr[0:64, kt * HID : (kt + 1) * HID] = w1f[kt * 64 : kt * 64 + 64, :]
    w1dr[64, 0:HID] = b1q
    w1dr[64, HID:] = b1r
    w1dr = w1dr.astype(FP8)

    w2b = np.asarray(W2).reshape(HID, 1)
    w2b = np.ascontiguousarray(
        np.stack([w2b[:128, 0], w2b[128:, 0]], axis=1)
    ).astype(FP16)  # [128, 2]

    plan = dict(
        npad=npad,
        nt=nt,
        ngroups=npad // GROUP_NODES,
        span=span,
        c0=c0,
        wdt=wdt,
        bounds=bounds,
        w1dr=w1dr,
        w2b=w2b,
    )
    return per_core, plan


def _legalize_waits(j):
    """Split multi-wait instructions: this container's walrus accepts at most
    one sync-wait per engine instruction. Hoist extras onto standalone
    EventSemaphore instructions (the same form raw-bass wait_ge produces)
    inserted immediately before, on the same engine."""
    n = 0
    for f in j["functions"]:
        for b in f["blocks"]:
            out = []
            for inst in b["instructions"]:
                si = inst.get("sync_info")
                ow = (si or {}).get("on_wait") or []
                if len(ow) > 1 and inst.get("opcode") != "EventSemaphore":
                    for w in ow[:-1]:
                        n += 1
                        out.append(
                            {
                                "debug": inst.get("debug", 0),
                                "engine": inst["engine"],
                                "ins": [],
                                "name": f"{inst['name']}_hw{n}",
                                "opcode": "EventSemaphore",
                                "outs": [],
                                "sync_info": {"on_update": [], "on_wait": [w]},
                            }
                        )
                    si["on_wait"] = [ow[-1]]
                out.append(inst)
            b["instructions"] = out
    return j


def _ensure_ntff_hook():
    import sys
    import types

    try:
        from antenv.axon_hooks import get_axon_ntff_profile_hook  # noqa: F401

        return
    except ImportError:
        pass
    from trn_agent_boot.trn_boot import _ntff_profile_via_ctypes

    hook = _ntff_profile_via_ctypes("/opt/axon/libaxon_pjrt.so")
    mod = types.ModuleType("antenv.axon_hooks")
    holder = {"hook": hook}
    mod.get_axon_ntff_profile_hook = lambda: holder["hook"]
    mod.set_axon_ntff_profile_hook = lambda h: holder.update(hook=h)
    import antenv

    antenv.axon_hooks = mod
    sys.modules["antenv.axon_hooks"] = mod


def _patch_serialization(nc):
    import json

    orig = nc.to_json_bytes

    def patched():
        j = json.loads(orig())
        _legalize_waits(j)
        return json.dumps(j).encode()

    nc.to_json_bytes = patched


def _build_program(plan):
    import concourse.bass as bass
    import concourse.mybir as mybir
    import concourse.tile as tile

    npad, nt, ngroups, span = plan["npad"], plan["nt"], plan["ngroups"], plan["span"]
    c0, wdt = plan["c0"], plan["wdt"]
    fp32 = mybir.dt.float32
    fp16 = mybir.dt.float16
    fp8 = mybir.dt.float8e4
    DR = mybir.MatmulPerfMode.DoubleRow

    nc = bass.Bass("TRN2", target_bir_lowering=True, debug=False)

    hT_d = nc.dram_tensor("hT", [65, 2 * npad], fp8, kind="ExternalInput").ap()
    hN_d = nc.dram_tensor(
        "hN", [TILE_NODES, nt * IN_DIM], fp16, kind="ExternalInput"
    ).ap()
    hmask = nc.dram_tensor(
        "hmask", [TILE_NODES, nt * span], fp16, kind="ExternalInput"
    ).ap()
    w1_d = nc.dram_tensor("W1", [65, 2 * HID], fp8, kind="ExternalInput").ap()
    w2_d = nc.dram_tensor("W2", [128, 2], fp16, kind="ExternalInput").ap()
    onum = nc.dram_tensor(
        "onum", [IN_DIM, NUM_BANK_COLS], fp32, kind="ExternalOutput"
    ).ap()
    ow = nc.dram_tensor("ow", [TILE_NODES, nt], fp32, kind="ExternalOutput").ap()

    silu = mybir.ActivationFunctionType.Silu
    mult = mybir.AluOpType.mult

    with tile.TileContext(nc) as tc:
        with (
            tc.tile_pool(name="consts", bufs=1) as consts,
            tc.tile_pool(name="ioN", bufs=8) as ioN,
            tc.tile_pool(name="ioT", bufs=4) as ioT,
            tc.tile_pool(name="smat", bufs=3) as smat,
            tc.tile_pool(name="little", bufs=3) as little,
            tc.tile_pool(name="upsum", bufs=2, space="PSUM") as upsum,
            tc.tile_pool(name="wpsum", bufs=1, space="PSUM") as wpsum,
            tc.tile_pool(name="npsum", bufs=1, space="PSUM") as npsum,
        ):
            w1_sb = consts.tile([65, 2 * HID], fp8)
            nc.sync.dma_start(w1_sb[:], w1_d[:])
            w2_sb = consts.tile([128, 2], fp16)
            nc.sync.dma_start(w2_sb[:], w2_d[:])
            mask_sb = consts.tile([TILE_NODES, nt * span], fp16)
            nc.sync.dma_start(mask_sb[:], hmask[:])

            # Pre-touch constants on their consuming engines so later ops
            # need only a single-engine sync wait (ISA wait-slot limits).
            preb = consts.tile([TILE_NODES, 1], fp16)
            nc.vector.tensor_copy(preb[:], mask_sb[:, 0:1])

            wall_sb = consts.tile([TILE_NODES, nt], fp32)
            e_sb = consts.tile([TILE_NODES, nt], fp32)
            num_ps = npsum.tile([IN_DIM, NUM_BANK_COLS], fp32)

            def w1_ap(half):
                base = w1_sb[:]
                return bass.AP(
                    base.tensor,
                    base.offset + half * 128,
                    [base.ap[0], [HID, 2], [1, 128]],
                )

            first_seg = True
            hN_tiles = {}
            for g in range(ngroups):
                hT_t = ioT.tile([65, 2 * GROUP_NODES], fp8, tag="hT")
                dst = hT_t[:]
                src = hT_d[:]
                nc.gpsimd.dma_start(
                    bass.AP(
                        dst.tensor,
                        dst.offset,
                        [dst.ap[0], [GROUP_NODES, 2], [1, GROUP_NODES]],
                    ),
                    bass.AP(
                        src.tensor,
                        src.offset + g * GROUP_NODES,
                        [src.ap[0], [npad, 2], [1, GROUP_NODES]],
                    ),
                )
                hN_t = ioN.tile([TILE_NODES, GROUP_NODES], fp16, tag="hN")
                nc.sync.dma_start(
                    hN_t[:], hN_d[:, g * GROUP_NODES : (g + 1) * GROUP_NODES]
                )
                hN_tiles[g] = hN_t

                # mm1 fp8 DoubleRow: u = [ua | ub] in one psum tile
                u = upsum.tile([128, 2 * GROUP_NODES], fp32, tag="u")
                for half in (0, 1):
                    for cs, ce in ((0, 512), (512, GROUP_NODES)):
                        rhs_b = hT_t[:]
                        rhs = bass.AP(
                            rhs_b.tensor,
                            rhs_b.offset + cs,
                            [rhs_b.ap[0], [GROUP_NODES, 2], [1, ce - cs]],
                        )
                        nc.tensor.matmul(
                            u[:, half * GROUP_NODES + cs : half * GROUP_NODES + ce],
                            w1_ap(half),
                            rhs,
                            start=True,
                            stop=True,
                            perf_mode=DR,
                        )

                # silu: one ACT instruction for the whole group (bias folded)
                s = smat.tile([128, 2 * GROUP_NODES], fp16, tag="s")
                nc.scalar.activation(s[:], u[:], silu)

                # mm2 fp16: per tile, 2 stationary-s matmuls into w psum
                w_ps = wpsum.tile([TILE_NODES, TPG], fp32, tag="w")
                for t in range(TPG):
                    ssl_a = slice(t * TILE_NODES, (t + 1) * TILE_NODES)
                    ssl_b = slice(
                        GROUP_NODES + t * TILE_NODES,
                        GROUP_NODES + (t + 1) * TILE_NODES,
                    )
                    nc.tensor.matmul(
                        w_ps[:, t : t + 1], s[:, ssl_a], w2_sb[:, 0:1],
                        start=True, stop=False,
                    )
                    nc.tensor.matmul(
                        w_ps[:, t : t + 1], s[:, ssl_b], w2_sb[:, 1:2],
                        start=False, stop=True,
                    )
                # export w for the host + exp batch (gpsimd)
                nc.vector.tensor_copy(
                    wall_sb[:, g * TPG : (g + 1) * TPG], w_ps[:]
                )

                if g % EXP_BATCH == EXP_BATCH - 1 or g == ngroups - 1:
                    gb = (g // EXP_BATCH) * EXP_BATCH
                    nb = (g + 1 - gb) * TPG
                    bsl = slice(gb * TPG, (g + 1) * TPG)
                    wsl = wall_sb[:, bsl]
                    sw = little.tile([TILE_NODES, EXP_BATCH * TPG], fp32, tag="sw")
                    nc.scalar.activation(sw[:, :nb], wsl, silu)
                    d_ = little.tile([TILE_NODES, EXP_BATCH * TPG], fp32, tag="d")
                    nc.gpsimd.tensor_sub(d_[:, :nb], wsl, sw[:, :nb])
                    r_ = little.tile([TILE_NODES, EXP_BATCH * TPG], fp32, tag="r")
                    nc.vector.reciprocal(r_[:, :nb], d_[:, :nb])
                    nc.gpsimd.tensor_mul(e_sb[:, bsl], sw[:, :nb], r_[:, :nb])

                    for gg in range(gb, g + 1):
                        S = smat.tile([TILE_NODES, TPG * span], fp16, tag="S")
                        eb = e_sb[:]
                        e_ap = bass.AP(
                            eb.tensor,
                            eb.offset + gg * TPG,
                            [eb.ap[0], [1, TPG], [0, span]],
                        )
                        msl = mask_sb[:, gg * TPG * span : (gg + 1) * TPG * span]
                        nc.vector.tensor_tensor(S[:], msl, e_ap, mult)
                        hN_gg = hN_tiles.pop(gg)
                        for tt in range(TPG):
                            t = gg * TPG + tt
                            col0, width = int(c0[t]), int(wdt[t])
                            fsl = slice(tt * IN_DIM, (tt + 1) * IN_DIM)
                            ssl2 = slice(tt * span, tt * span + width)
                            ncol = slice(col0, col0 + width)
                            nc.tensor.matmul(
                                num_ps[:, ncol], hN_gg[:, fsl], S[:, ssl2],
                                start=first_seg, stop=False,
                            )
                            first_seg = False

            nc.sync.dma_start(ow[:], wall_sb[:])
            num_sb = consts.tile([IN_DIM, NUM_BANK_COLS], fp32)
            nc.vector.tensor_copy(num_sb[:], num_ps[:])
            nc.sync.dma_start(onum[:], num_sb[:])

    return nc


def kernel(h, batch, W1, b1, W2, b2):
    h = np.asarray(h, dtype=np.float32)
    batch = np.asarray(batch)
    W1 = np.asarray(W1, dtype=np.float32)
    b1 = np.asarray(b1, dtype=np.float32)
    W2 = np.asarray(W2, dtype=np.float32)
    b2 = np.asarray(b2, dtype=np.float32)

    per_core, plan = _build_host_data(h, batch, W1, b1, W2)
    nc = _build_program(plan)

    from concourse.bass_utils import run_bass_kernel_spmd

    in_maps = []
    for c in range(NCORES):
        pc = per_core[c]
        in_maps.append(
            {
                "hT": pc["hT"],
                "hN": pc["hN"],
                "hmask": pc["hmask"],
                "W1": plan["w1dr"],
                "W2": plan["w2b"],
            }
        )
    _patch_serialization(nc)
    import os
    import time as _time
    trace = bool(os.environ.get("ATT_TRACE"))
    res = None
    if trace:
        # NTFF profile of device 0; the gauge post-processing in this
        # container lacks some tools, so parse the raw ntff json ourselves.
        import glob
        import json as _json
        import tempfile

        _ensure_ntff_hook()
        import concourse.bass_utils as _bu

        _bu.upload_artifacts = lambda d: d  # no bucket in this container
        tdir = os.environ.get("ATT_TRACE_DIR") or tempfile.mkdtemp()
        try:
            res = run_bass_kernel_spmd(
                nc, in_maps, list(range(NCORES)), trace=True, tmpdir=tdir
            )
        except Exception:
            res = None  # post-processing crash; ntff json may still exist
        for f in sorted(glob.glob(os.path.join(tdir, "ntff_*.json"))):
            try:
                s = _json.load(open(f))["summary"]
                if isinstance(s, list):
                    s = s[0]
                print(f"HW exec time: {s['total_time'] * 1e9:.0f} ns")
                break
            except Exception:
                pass
    if res is None:
        res = run_bass_kernel_spmd(nc, in_maps, list(range(NCORES)))
    nbench = int(os.environ.get("ATT_BENCH", "0"))
    if nbench:
        times = []
        for _ in range(nbench):
            t0 = _time.perf_counter()
            res = run_bass_kernel_spmd(nc, in_maps, list(range(NCORES)))
            times.append(_time.perf_counter() - t0)
        best = min(times)
        print(f"exec wall (best of {nbench}): {best*1e3:.2f} ms  "
              f"(times: {[f'{t*1e3:.1f}' for t in times]})")

    # Host: den from w, global max, final divide, assemble.
    out = np.empty((G_TOTAL, IN_DIM), np.float32)
    m_glob = -np.inf
    core_data = []
    for c in range(NCORES):
        r = res.results[c]
        w_flat = np.asarray(r["ow"]).T.reshape(-1)[: per_core[c]["n_nodes"]]
        m_glob = max(m_glob, float(w_flat.max()))
        core_data.append((np.asarray(r["onum"]), w_flat))
    for c in range(NCORES):
        onum_a, w_flat = core_data[c]
        e = np.exp(w_flat.astype(np.float64))
        den = np.bincount(
            per_core[c]["grel"], weights=e, minlength=G_PER_CORE
        )[:G_PER_CORE]
        den = den + EPS * math.exp(m_glob)
        out[c * G_PER_CORE : (c + 1) * G_PER_CORE] = (
            onum_a[:, :G_PER_CORE] / den[None, :].astype(np.float32)
        ).T
    return out


# revision 12
# speedup vs baseline: 1.0452x; 1.0452x over previous
"""AttentionPool Trainium2 kernel (8 NeuronCores, SPMD, no collectives).

Math (exactly equivalent to the reference up to fp reordering):
    w_i   = silu(h_i @ W1 + b1) @ W2          (b2 cancels)
    num_g = sum_{i in g} h_i * exp(w_i)
    den_g = sum_{i in g} exp(w_i)
    out_g = num_g / (den_g + eps * exp(max_j w_j))

v2 design (per 768-node group, 82 groups/core):
    mm1:  u[256, n] via fp8e4m3 DoubleRow matmuls, K = 2 k-tiles x 65
          partitions (64 features + a ones-row that carries b1 in the
          stationary, two-level-quantized across the two k-tile slots)
    silu: ONE ACT instruction [128, 1536] PSUM->SBUF fp16 (bias folded)
    mm2:  per-tile stationary-s matmuls, fp16 (12 tiny matmuls/group)
    exp:  e = silu(w)/(w - silu(w)) == exp(w), batched over 4 groups,
          sub/mult/copy offloaded to GpSimd, recip on DVE
    S:    S[p, c] = e_p * (batch_rel_p == c)   (DVE tensor_tensor)
    seg:  num[feat, g] += hN_tile^T @ S        (PE, accumulated in PSUM)

DMA: hN fp16 (sync queue) + hT fp8 (gpsimd queue) + mask fp16 -> ~25MB/core
(vs 34MB baseline), split across two hw DMA queues.

Host: shards nodes at graph boundaries (512 graphs/core), computes den/max/
final divide from the returned per-node logits w.
"""

import math

import ml_dtypes
import numpy as np

NCORES = 8
G_TOTAL = 4096
G_PER_CORE = G_TOTAL // NCORES  # 512
IN_DIM = 128
HID = 256
EPS = 1e-6
GROUP_NODES = 768
TILE_NODES = 128
TPG = GROUP_NODES // TILE_NODES  # 6
NUM_BANK_COLS = 512  # one PSUM bank of f32
EXP_BATCH = 4  # groups per exp-trick batch

BF16 = ml_dtypes.bfloat16
FP16 = np.float16
FP8 = ml_dtypes.float8_e4m3


def _tilepart(a, nt):
    return a.reshape(nt, TILE_NODES, IN_DIM).transpose(1, 0, 2).reshape(
        TILE_NODES, nt * IN_DIM
    )


def _build_host_data(h, batch, W1, b1, W2):
    """Shard at graph boundaries; build per-core arrays + global window plan."""
    batch = np.asarray(batch).astype(np.int64)
    cnt = np.bincount(batch, minlength=G_TOTAL)
    cum = np.concatenate([[0], np.cumsum(cnt)])
    bounds = [int(cum[G_PER_CORE * c]) for c in range(NCORES + 1)]
    sizes = np.diff(bounds)
    npad = int(math.ceil(max(sizes) / GROUP_NODES) * GROUP_NODES)
    nt = npad // TILE_NODES

    # Global (core-invariant) window starts: c0[t] = min over cores of the
    # first graph (relative) in tile t; SPAN covers the max extent.
    lo = np.full(nt, 1 << 30, dtype=np.int64)
    hi = np.full(nt, -1, dtype=np.int64)
    grels = []
    for c in range(NCORES):
        n0, n1 = bounds[c], bounds[c + 1]
        grel = batch[n0:n1] - G_PER_CORE * c
        grels.append(grel)
        ntc = (n1 - n0 + TILE_NODES - 1) // TILE_NODES
        for t in range(ntc):
            seg = grel[TILE_NODES * t : TILE_NODES * t + TILE_NODES]
            lo[t] = min(lo[t], int(seg[0]))
            hi[t] = max(hi[t], int(seg[-1]))
    span = 8
    while span < int(max(hi - lo)) + 1:
        span *= 2
    assert span <= 64, f"window span {span} unexpectedly large"
    c0 = np.where(hi >= 0, lo, 0).astype(np.int64)
    c0 = np.minimum(c0, G_PER_CORE - 1)  # clamp (padding tiles)
    # fill padding tiles' c0 with last valid to keep windows sane
    last = 0
    for t in range(nt):
        if hi[t] >= 0:
            last = c0[t]
        else:
            c0[t] = last
    wdt = np.minimum(span, G_PER_CORE - c0).astype(np.int64)  # clip to 512

    per_core = []
    for c in range(NCORES):
        n0, n1 = bounds[c], bounds[c + 1]
        nc_nodes = n1 - n0
        hc = np.empty((npad, IN_DIM), np.float32)
        hc[:nc_nodes] = h[n0:n1]
        hc[nc_nodes:] = h[n0]  # replicate a real node into padding
        # hT fp8 DoubleRow layout: [65, 2*npad], k-tile kt holds feats
        # kt*64 + p on partition p; partition 64 is the ones-row (bias).
        ht = np.ascontiguousarray(hc.T)  # [128, npad] fp32
        hT8 = np.empty((65, 2 * npad), np.float32)
        hT8[0:64, 0:npad] = ht[0:64]
        hT8[0:64, npad:] = ht[64:128]
        hT8[64, :] = 1.0
        brel = np.full(npad, -1000.0, np.float32)
        g = grels[c].astype(np.float32)
        tidx = np.arange(nc_nodes) // TILE_NODES
        brel[:nc_nodes] = g - c0[tidx]
        per_core.append(
            dict(
                hN=_tilepart(hc.astype(FP16), nt),
                hT=hT8.astype(FP8),
                hmask=np.ascontiguousarray(
                    (
                        brel.reshape(nt, TILE_NODES).T[:, :, None]
                        == np.arange(span, dtype=np.float32)[None, None, :]
                    )
                    .astype(FP16)
                    .reshape(TILE_NODES, nt * span)
                ),
                n_nodes=nc_nodes,
                grel=grels[c],
            )
        )

    # W1 fp8 DoubleRow stationary: [65, 512], free = kt*256 + hid.
    # Row 64 carries b1 two-level-quantized: kt0 slot the fp8 rounding of
    # b1, kt1 slot the fp8 rounding of the residual.
    w1f = np.asarray(W1).astype(np.float32)  # [128, 256]
    b1f = np.asarray(b1).reshape(HID).astype(np.float32)
    b1q = b1f.astype(FP8).astype(np.float32)
    b1r = b1f - b1q
    w1dr = np.zeros((65, 2 * HID), np.float32)
    for kt in range(2):
        w1dr[0:64, kt * HID : (kt + 1) * HID] = w1f[kt * 64 : kt * 64 + 64, :]
    w1dr[64, 0:HID] = b1q
    w1dr[64, HID:] = b1r
    w1dr = w1dr.astype(FP8)

    w2b = np.asarray(W2).reshape(HID, 1)
    w2b = np.ascontiguousarray(
        np.stack([w2b[:128, 0], w2b[128:, 0]], axis=1)
    ).astype(FP8)  # [128, 2], fp8 for DoubleRow mm2

    plan = dict(
        npad=npad,
        nt=nt,
        ngroups=npad // GROUP_NODES,
        span=span,
        c0=c0,
        wdt=wdt,
        bounds=bounds,
        w1dr=w1dr,
        w2b=w2b,
    )
    return per_core, plan


def _legalize_waits(j):
    """Split multi-wait instructions: this container's walrus accepts at most
    one sync-wait per engine instruction. Hoist extras onto standalone
    EventSemaphore instructions (the same form raw-bass wait_ge produces)
    inserted immediately before, on the same engine."""
    n = 0
    for f in j["functions"]:
        for b in f["blocks"]:
            out = []
            for inst in b["instructions"]:
                si = inst.get("sync_info")
                ow = (si or {}).get("on_wait") or []
                if len(ow) > 1 and inst.get("opcode") != "EventSemaphore":
                    for w in ow[:-1]:
                        n += 1
                        out.append(
                            {
                                "debug": inst.get("debug", 0),
                                "engine": inst["engine"],
                                "ins": [],
                                "name": f"{inst['name']}_hw{n}",
                                "opcode": "EventSemaphore",
                                "outs": [],
                                "sync_info": {"on_update": [], "on_wait": [w]},
                            }
                        )
                    si["on_wait"] = [ow[-1]]
                out.append(inst)
            b["instructions"] = out
    return j


def _ensure_ntff_hook():
    import sys
    import types

    try:
        from antenv.axon_hooks import get_axon_ntff_profile_hook  # noqa: F401

        return
    except ImportError:
        pass
    from trn_agent_boot.trn_boot import _ntff_profile_via_ctypes

    hook = _ntff_profile_via_ctypes("/opt/axon/libaxon_pjrt.so")
    mod = types.ModuleType("antenv.axon_hooks")
    holder = {"hook": hook}
    mod.get_axon_ntff_profile_hook = lambda: holder["hook"]
    mod.set_axon_ntff_profile_hook = lambda h: holder.update(hook=h)
    import antenv

    antenv.axon_hooks = mod
    sys.modules["antenv.axon_hooks"] = mod


def _patch_serialization(nc):
    import json

    orig = nc.to_json_bytes

    def patched():
        j = json.loads(orig())
        _legalize_waits(j)
        return json.dumps(j).encode()

    nc.to_json_bytes = patched


def _build_program(plan):
    import concourse.bass as bass
    import concourse.mybir as mybir
    import concourse.tile as tile

    npad, nt, ngroups, span = plan["npad"], plan["nt"], plan["ngroups"], plan["span"]
    c0, wdt = plan["c0"], plan["wdt"]
    fp32 = mybir.dt.float32
    fp16 = mybir.dt.float16
    fp8 = mybir.dt.float8e4
    DR = mybir.MatmulPerfMode.DoubleRow

    nc = bass.Bass("TRN2", target_bir_lowering=True, debug=False)

    hT_d = nc.dram_tensor("hT", [65, 2 * npad], fp8, kind="ExternalInput").ap()
    hN_d = nc.dram_tensor(
        "hN", [TILE_NODES, nt * IN_DIM], fp16, kind="ExternalInput"
    ).ap()
    hmask = nc.dram_tensor(
        "hmask", [TILE_NODES, nt * span], fp16, kind="ExternalInput"
    ).ap()
    w1_d = nc.dram_tensor("W1", [65, 2 * HID], fp8, kind="ExternalInput").ap()
    w2_d = nc.dram_tensor("W2", [128, 2], fp8, kind="ExternalInput").ap()
    onum = nc.dram_tensor(
        "onum", [IN_DIM, NUM_BANK_COLS], fp32, kind="ExternalOutput"
    ).ap()
    ow = nc.dram_tensor("ow", [TILE_NODES, nt], fp32, kind="ExternalOutput").ap()

    silu = mybir.ActivationFunctionType.Silu
    mult = mybir.AluOpType.mult

    DB = 4  # groups per batched DMA
    nb = (ngroups + DB - 1) // DB
    SEG_LAG = EXP_BATCH + 1  # groups by which seg trails mm1
    PREFETCH = 2  # DMA batches issued ahead

    with tile.TileContext(nc) as tc:
        with (
            tc.tile_pool(name="consts", bufs=1) as consts,
            tc.tile_pool(name="ioN", bufs=5) as ioN,
            tc.tile_pool(name="ioT", bufs=5) as ioT,
            tc.tile_pool(name="smat", bufs=3) as smat,
            tc.tile_pool(name="sS", bufs=3) as sS,
            tc.tile_pool(name="little", bufs=3) as little,
            tc.tile_pool(name="upsum", bufs=2, space="PSUM") as upsum,
            tc.tile_pool(name="wpsum", bufs=1, space="PSUM") as wpsum,
            tc.tile_pool(name="npsum", bufs=1, space="PSUM") as npsum,
        ):
            w1_sb = consts.tile([65, 2 * HID], fp8)
            nc.sync.dma_start(w1_sb[:], w1_d[:])
            w2_sb = consts.tile([128, 2], fp8)
            nc.sync.dma_start(w2_sb[:], w2_d[:])
            mask_sb = consts.tile([TILE_NODES, nt * span], fp16)
            nc.sync.dma_start(mask_sb[:], hmask[:])

            # Pre-touch the mask on its consuming engine (single-wait rule).
            preb = consts.tile([TILE_NODES, 1], fp16)
            nc.vector.tensor_copy(preb[:], mask_sb[:, 0:1])

            wall_sb = consts.tile([TILE_NODES, nt], fp32)
            e_sb = consts.tile([TILE_NODES, nt], fp32)
            num_ps = npsum.tile([IN_DIM, NUM_BANK_COLS], fp32)

            def w1_ap(half):
                base = w1_sb[:]
                return bass.AP(
                    base.tensor,
                    base.offset + half * 128,
                    [base.ap[0], [HID, 2], [1, 128]],
                )

            w2_ap = bass.AP(
                w2_sb[:].tensor, w2_sb[:].offset, [w2_sb[:].ap[0], [1, 2], [1, 1]]
            )

            io_tilesN = {}
            io_tilesT = {}

            def issue_dma(b):
                g0 = b * DB
                gn = min(DB, ngroups - g0)
                cols = gn * GROUP_NODES
                hN_t = ioN.tile([TILE_NODES, DB * GROUP_NODES], fp16, tag="hN")
                nc.sync.dma_start(
                    hN_t[:, :cols],
                    hN_d[:, g0 * GROUP_NODES : g0 * GROUP_NODES + cols],
                )
                hT_t = ioT.tile([65, 2 * DB * GROUP_NODES], fp8, tag="hT")
                dst = hT_t[:]
                src = hT_d[:]
                nc.gpsimd.dma_start(
                    bass.AP(
                        dst.tensor,
                        dst.offset,
                        [dst.ap[0], [DB * GROUP_NODES, 2], [1, cols]],
                    ),
                    bass.AP(
                        src.tensor,
                        src.offset + g0 * GROUP_NODES,
                        [src.ap[0], [npad, 2], [1, cols]],
                    ),
                )
                io_tilesN[b] = hN_t
                io_tilesT[b] = hT_t

            def mm1(g):
                hT_t = io_tilesT[g // DB]
                goff = (g % DB) * GROUP_NODES
                u = upsum.tile([128, 2 * GROUP_NODES], fp32, tag="u")
                base = hT_t[:]
                for half in (0, 1):
                    for cs, ce in ((0, 512), (512, GROUP_NODES)):
                        rhs = bass.AP(
                            base.tensor,
                            base.offset + goff + cs,
                            [base.ap[0], [DB * GROUP_NODES, 2], [1, ce - cs]],
                        )
                        nc.tensor.matmul(
                            u[:, half * GROUP_NODES + cs : half * GROUP_NODES + ce],
                            w1_ap(half),
                            rhs,
                            start=True,
                            stop=True,
                            perf_mode=DR,
                        )
                return u

            def do_silu(u):
                s = smat.tile([128, 2 * GROUP_NODES], fp8, tag="s")
                nc.scalar.activation(s[:], u[:], silu)
                return s

            def mm2(g, s):
                # DoubleRow over the two hidden halves: K = 256 in one pass
                w_ps = wpsum.tile([TILE_NODES, TPG], fp32, tag="w")
                sb = s[:]
                for t in range(TPG):
                    lhsT = bass.AP(
                        sb.tensor,
                        sb.offset + t * TILE_NODES,
                        [sb.ap[0], [GROUP_NODES, 2], [1, TILE_NODES]],
                    )
                    nc.tensor.matmul(
                        w_ps[:, t : t + 1], lhsT, w2_ap, start=True, stop=True,
                        perf_mode=DR,
                    )
                nc.vector.tensor_copy(
                    wall_sb[:, g * TPG : (g + 1) * TPG], w_ps[:]
                )

            def exp_batch(g_last):
                gb = (g_last // EXP_BATCH) * EXP_BATCH
                nbc = (g_last + 1 - gb) * TPG
                bsl = slice(gb * TPG, (g_last + 1) * TPG)
                wsl = wall_sb[:, bsl]
                sw = little.tile([TILE_NODES, EXP_BATCH * TPG], fp32, tag="sw")
                nc.scalar.activation(sw[:, :nbc], wsl, silu)
                d_ = little.tile([TILE_NODES, EXP_BATCH * TPG], fp32, tag="d")
                nc.vector.tensor_sub(d_[:, :nbc], wsl, sw[:, :nbc])
                # guard the w == 0 singularity of the exp trick:
                # dd = d + (d==0); e = sw/dd + (d==0)  (exp(0) == 1 exactly)
                iseq = mybir.AluOpType.is_equal
                add = mybir.AluOpType.add
                dd = little.tile([TILE_NODES, EXP_BATCH * TPG], fp32, tag="dd")
                nc.vector.scalar_tensor_tensor(
                    dd[:, :nbc], d_[:, :nbc], 0.0, d_[:, :nbc], iseq, add
                )
                r_ = little.tile([TILE_NODES, EXP_BATCH * TPG], fp32, tag="r")
                nc.vector.reciprocal(r_[:, :nbc], dd[:, :nbc])
                e0 = little.tile([TILE_NODES, EXP_BATCH * TPG], fp32, tag="e0")
                nc.vector.tensor_mul(e0[:, :nbc], sw[:, :nbc], r_[:, :nbc])
                nc.vector.scalar_tensor_tensor(
                    e_sb[:, bsl], d_[:, :nbc], 0.0, e0[:, :nbc], iseq, add
                )

            first_seg = [True]

            def seg(gg):
                S = sS.tile([TILE_NODES, TPG * span], fp16, tag="S")
                eb = e_sb[:]
                e_ap = bass.AP(
                    eb.tensor,
                    eb.offset + gg * TPG,
                    [eb.ap[0], [1, TPG], [0, span]],
                )
                msl = mask_sb[:, gg * TPG * span : (gg + 1) * TPG * span]
                nc.vector.tensor_tensor(S[:], msl, e_ap, mult)
                hN_t = io_tilesN[gg // DB]
                goff = (gg % DB) * GROUP_NODES
                for tt in range(TPG):
                    t = gg * TPG + tt
                    col0, width = int(c0[t]), int(wdt[t])
                    fsl = slice(goff + tt * IN_DIM, goff + (tt + 1) * IN_DIM)
                    ssl2 = slice(tt * span, tt * span + width)
                    ncol = slice(col0, col0 + width)
                    nc.tensor.matmul(
                        num_ps[:, ncol], hN_t[:, fsl], S[:, ssl2],
                        start=first_seg[0], stop=False,
                    )
                    first_seg[0] = False

            # Software-pipelined main loop: mm1/silu lead, mm2 lags 1 group,
            # seg lags SEG_LAG groups (after its exp batch resolved).
            issue_dma(0)
            issue_dma(1)
            u_of = {}
            s_of = {}
            for g in range(ngroups + SEG_LAG):
                if g < ngroups:
                    if g % DB == 0 and (b := g // DB + PREFETCH) < nb:
                        issue_dma(b)
                    u_of[g] = mm1(g)
                    s_of[g] = do_silu(u_of.pop(g))
                gm = g - 1
                if 0 <= gm < ngroups:
                    mm2(gm, s_of.pop(gm))
                    if gm % EXP_BATCH == EXP_BATCH - 1 or gm == ngroups - 1:
                        exp_batch(gm)
                gs = g - SEG_LAG
                if 0 <= gs < ngroups:
                    seg(gs)

            nc.sync.dma_start(ow[:], wall_sb[:])
            num_sb = consts.tile([IN_DIM, NUM_BANK_COLS], fp32)
            nc.vector.tensor_copy(num_sb[:], num_ps[:])
            nc.sync.dma_start(onum[:], num_sb[:])

    return nc


def kernel(h, batch, W1, b1, W2, b2):
    h = np.asarray(h, dtype=np.float32)
    batch = np.asarray(batch)
    W1 = np.asarray(W1, dtype=np.float32)
    b1 = np.asarray(b1, dtype=np.float32)
    W2 = np.asarray(W2, dtype=np.float32)
    b2 = np.asarray(b2, dtype=np.float32)

    per_core, plan = _build_host_data(h, batch, W1, b1, W2)
    nc = _build_program(plan)

    from concourse.bass_utils import run_bass_kernel_spmd

    in_maps = []
    for c in range(NCORES):
        pc = per_core[c]
        in_maps.append(
            {
                "hT": pc["hT"],
                "hN": pc["hN"],
                "hmask": pc["hmask"],
                "W1": plan["w1dr"],
                "W2": plan["w2b"],
            }
        )
    _patch_serialization(nc)
    import os
    import time as _time
    trace = bool(os.environ.get("ATT_TRACE"))
    res = None
    if trace:
        # NTFF profile of device 0; the gauge post-processing in this
        # container lacks some tools, so parse the raw ntff json ourselves.
        import glob
        import json as _json
        import tempfile

        _ensure_ntff_hook()
        import concourse.bass_utils as _bu

        _bu.upload_artifacts = lambda d: d  # no bucket in this container
        tdir = os.environ.get("ATT_TRACE_DIR") or tempfile.mkdtemp()
        try:
            res = run_bass_kernel_spmd(
                nc, in_maps, list(range(NCORES)), trace=True, tmpdir=tdir
            )
        except Exception:
            res = None  # post-processing crash; ntff json may still exist
        for f in sorted(glob.glob(os.path.join(tdir, "ntff_*.json"))):
            try:
                s = _json.load(open(f))["summary"]
                if isinstance(s, list):
                    s = s[0]
                print(f"HW exec time: {s['total_time'] * 1e9:.0f} ns")
                break
            except Exception:
                pass
    if res is None:
        res = run_bass_kernel_spmd(nc, in_maps, list(range(NCORES)))
    nbench = int(os.environ.get("ATT_BENCH", "0"))
    if nbench:
        times = []
        for _ in range(nbench):
            t0 = _time.perf_counter()
            res = run_bass_kernel_spmd(nc, in_maps, list(range(NCORES)))
            times.append(_time.perf_counter() - t0)
        best = min(times)
        print(f"exec wall (best of {nbench}): {best*1e3:.2f} ms  "
              f"(times: {[f'{t*1e3:.1f}' for t in times]})")

    # Host: den from w, global max, final divide, assemble.
    out = np.empty((G_TOTAL, IN_DIM), np.float32)
    m_glob = -np.inf
    core_data = []
    for c in range(NCORES):
        r = res.results[c]
        w_flat = np.asarray(r["ow"]).T.reshape(-1)[: per_core[c]["n_nodes"]]
        m_glob = max(m_glob, float(w_flat.max()))
        core_data.append((np.asarray(r["onum"]), w_flat))
    for c in range(NCORES):
        onum_a, w_flat = core_data[c]
        e = np.exp(w_flat.astype(np.float64))
        den = np.bincount(
            per_core[c]["grel"], weights=e, minlength=G_PER_CORE
        )[:G_PER_CORE]
        den = den + EPS * math.exp(m_glob)
        out[c * G_PER_CORE : (c + 1) * G_PER_CORE] = (
            onum_a[:, :G_PER_CORE] / den[None, :].astype(np.float32)
        ).T
    return out
